# revision 24
# baseline (speedup 1.0000x reference)
"""MultiHeadAttention kernel for 8x TRN2 NeuronCores.

The reference module's einsum reduces the attention tensor over BOTH the
query and key axes (attn_mass = sum_{q,k} softmax(logits)_k), and softmax
rows sum to 1, so attn_mass == Lq exactly for every (batch, head).  The
whole computation collapses to a single dense GEMM after folding the
(block-diagonal) per-head V-projection into the output projection:

    out = V_flat @ W_eff + b_eff          (4096 x 1024) @ (1024 x 1024)
    W_eff[h*hd+a, n] = Lq * sum_b Wv[b, a] * Wo[n, h*hd+b]
    b_eff[n]         = Lq * sum_{h,b} Wo[n, h*hd+b] * bv[b] + bo[n]

Row-sharded across 8 cores (512 rows each), computed TRANSPOSED so the
bias is a per-partition scalar folded into the PSUM eviction.

v5: X and bank-0's W stream as fp8 E3M4 (W0 pre-scaled by 1/8 to fit
E3M4's +-15.5 range; bank 0 evicts with a fused x8).  The PE upconverts
operands to fp22, so mixed bf16 x e3m4 matmuls are exact on the
quantized values (HW-verified); total quantization error vs the f64
reference: l2 1.24e-2, scale-absmax 1.44e-2 (gate 2e-2).  fp8 X halves
the bandwidth-critical early bytes -- the two HWDGE queues together
sustain only ~320 B/ns and the early phase (X + first W banks) bounds
when the PE can run dense.

NTFF-measured framework constants this schedule is built around:
window-start -> first dispatch ~1.1us, first bytes ~2.6us (sync) /
~3.5us (scalar), ~0.6us dispatch per DMA (engine-serial), completion
semaphore ~0.9us, teardown ring ~8.9us after the last output byte (all
fixed).  DMA needs >=2KB per-partition lines for full rate; the PE is
in-order so emission order must match operand arrival; Tile deps are
tile-granular (never split a psum bank's matmuls around its eviction).

  * sync:   c1=[W0 k0-3 | X0 | X1] (fp8) -> c2=[X4..X7] (fp8) -> bias
            -> W2 -> W4 -> W6     (~1.2MB)
    scalar: d1=[W0 k4-7 | X2 | X3] (fp8) -> W1 -> W3 -> W5 -> W7
  * emission: junk warm-up (HAM clock gate needs ~3.4us of PE busy),
    then b0 k-pieces as the three fp8 chunks land, then banks 1..7
    dense in W-arrival order; evictions (DVE, bias fused) pipeline
    behind each bank with output DMAs alternating queues;
  * tail: bank 7 stays full-width; its eviction splits DVE 288 /
    ACT 224 into separate ob tiles with DMAs on both queues.
"""

import numpy as np
import ml_dtypes

import concourse.bass as bass
import concourse.bacc as bacc
import concourse.mybir as mybir
from concourse.tile import TileContext
from concourse.bass_utils import run_bass_kernel_spmd

N_CORES = 8
E = 1024            # embed dim == d_model
H, HD = 16, 64      # heads, head dim
ROWS = 4096         # N * L = 2 * 2048
RPC = ROWS // N_CORES   # rows per core = 512
P = 128             # SBUF partitions
KT = E // P         # 8 contraction slabs
JT = E // P         # 8 output-column banks

N_WARM = 6          # junk matmuls before the first real matmul
JF = 512            # junk matmul free dim (warmup)
JF2 = 256           # filler junk free dim
SPL = 320           # bank-7 eviction split point (two DVE pieces)
W0S = 8.0           # bank-0 weight pre-scale (E3M4 max 15.5, |W0| < 99)

BF16 = ml_dtypes.bfloat16
F8E3 = ml_dtypes.float8_e3m4

_NC_CACHE = {}
LAST_RESULTS = None  # BassKernelResults of the most recent device run


def _build():
    f32 = mybir.dt.float32
    bf = mybir.dt.bfloat16
    e3 = mybir.dt.float8e3
    nc = bacc.Bacc(None, target_bir_lowering=False)

    def dram(name, cols, dt):
        return nc.declare_dram_parameter(name, [P, cols], dt, isOutput=False)

    # sync-queue stream (first-use order)
    c1 = dram("c1", 4 * P + RPC, e3)  # [W0 k0-3 (/8) | X0] fast-start
    c2 = dram("c2", RPC, e3)        # X1
    c6 = dram("c6", 4 * RPC, e3)    # [X4 | X5 | X6 | X7]
    cb = dram("cb", JT, f32)        # bias
    c3 = dram("c3", E, bf)          # W2
    c4 = dram("c4", E, bf)          # W4
    c5 = dram("c5", E, bf)          # W6
    # scalar-queue stream
    d1 = dram("d1", 3 * RPC, e3)    # [W0 k4-7 (/8) | X2 | X3]
    d2 = dram("d2", E, bf)          # W1
    d3 = dram("d3", E, bf)          # W3
    d4 = dram("d4", E, bf)          # W5
    d5 = dram("d5", E, bf)          # W7
    outp = nc.declare_dram_parameter("outp", [P, JT * RPC], bf, isOutput=True)

    with TileContext(nc) as tc:
        with (
            tc.tile_pool(name="ip", bufs=1) as ip,
            tc.tile_pool(name="pp", bufs=1, space="PSUM") as pp,
            tc.tile_pool(name="op", bufs=1) as op,
        ):
            cap = nc.const_aps.aps[(bf, 1.0)]
            cb_l = cap.broadcast_to([P, P])
            cb_r = {JF: cap.broadcast_to([P, JF]), JF2: cap.broadcast_to([P, JF2])}

            cols = {"c1": 4 * P + RPC, "c2": RPC, "c6": 4 * RPC, "cb": JT,
                    "c3": E, "c4": E, "c5": E, "d1": 3 * RPC, "d2": E,
                    "d3": E, "d4": E, "d5": E}
            dts = {"c1": e3, "c2": e3, "c6": e3, "cb": f32, "d1": e3}
            T = {}
            for prm in (c1, c2, c6, cb, c3, c4, c5, d1, d2, d3, d4, d5):
                n = prm.name
                T[n] = ip.tile([P, cols[n]], dts.get(n, bf), name=n, tag=n)

            for prm in (c1, c2, c6, cb, c3, c4, c5):
                nc.sync.dma_start(out=T[prm.name][:], in_=prm[:, :])
            for prm in (d1, d2, d3, d4, d5):
                nc.scalar.dma_start(out=T[prm.name][:], in_=prm[:, :])

            bias_t = T["cb"]
            WT = {1: T["d2"], 2: T["c3"], 3: T["d3"], 4: T["c4"],
                  5: T["d4"], 6: T["c5"], 7: T["d5"]}

            def rhs(k, c0=0, c1=RPC):
                if k == 0:
                    t, o = T["c1"], 4 * P
                elif k == 1:
                    t, o = T["c2"], 0
                elif k < 4:
                    t, o = T["d1"], (k - 1) * RPC
                else:
                    t, o = T["c6"], (k - 4) * RPC
                return t[:, o + c0:o + c1]

            def lhsT(j, k):
                if j == 0:
                    t = T["c1"] if k < 4 else T["d1"]
                    return t[:, (k % 4) * P:(k % 4 + 1) * P]
                return WT[j][:, k * P:(k + 1) * P]

            ps = [
                pp.tile([P, RPC], f32, name=f"ps{j}", tag=f"ps{j}")
                for j in range(JT)
            ]
            obs = [
                op.tile([P, RPC], bf, name=f"ob{j}", tag=f"ob{j}")
                for j in range(JT - 1)
            ]
            ob7a = op.tile([P, SPL], bf, name="ob7a", tag="ob7a")
            ob7b = op.tile([P, RPC - SPL], bf, name="ob7b", tag="ob7b")

            def junk(i, f=JF2):
                nc.tensor.matmul(
                    ps[i][:, 0:f], cb_l, cb_r[f], start=True, stop=True,
                )

            for i in range(N_WARM):
                junk(7 - (i % 2), JF)   # ps7/ps6: real start comes latest

            def mm(j, k, start=False, stop=False):
                nc.tensor.matmul(
                    ps[j], lhsT(j, k), rhs(k), start=start, stop=stop,
                )

            def evict(j, eng):
                if j == 0:
                    # bank 0's W was pre-scaled by 1/W0S for E3M4 range
                    nc.vector.tensor_scalar(
                        obs[0][:], ps[0], W0S, bias_t[:, 0:1],
                        mybir.AluOpType.mult, mybir.AluOpType.add,
                    )
                else:
                    nc.vector.tensor_scalar_add(
                        obs[j][:], ps[j], bias_t[:, j:j + 1]
                    )
                eng.dma_start(
                    out=outp[:, j * RPC:(j + 1) * RPC], in_=obs[j][:]
                )

            # bank 0 piece-by-piece as the fp8 chunks land (c1 fast-start
            # ~4.5us, X1 ~5.0, d1 ~5.6, c6 ~6.6), then bank 1 when W1
            # arrives (~7.2), then banks 2..6 in W-arrival order.
            mm(0, 0, start=True)
            junk(6)
            mm(0, 1)
            junk(6)
            mm(0, 2)
            mm(0, 3)
            junk(6)
            for k in range(4, KT):
                mm(0, k, stop=(k == KT - 1))
            evict(0, nc.sync)
            mm(1, 0, start=True)
            for k in range(1, KT):
                mm(1, k, stop=(k == KT - 1))
            evict(1, nc.scalar)
            # banks 2..6 dense in W-arrival order
            engs = [nc.sync, nc.scalar]
            for n, j in enumerate([2, 3, 4, 5, 6]):
                mm(j, 0, start=True)
                for k in range(1, KT):
                    mm(j, k, stop=(k == KT - 1))
                evict(j, engs[n % 2])
            # bank 7 full-width; eviction split in two DVE pieces (ACT has
            # a ~0.6us start lag) with DMAs on both queues
            mm(7, 0, start=True)
            for k in range(1, KT):
                mm(7, k, stop=(k == KT - 1))
            nc.vector.tensor_scalar_add(
                ob7a[:], ps[7][:, 0:SPL], bias_t[:, 7:8]
            )
            nc.sync.dma_start(
                out=outp[:, 7 * RPC:7 * RPC + SPL], in_=ob7a[:]
            )
            nc.vector.tensor_scalar_add(
                ob7b[:], ps[7][:, SPL:RPC], bias_t[:, 7:8]
            )
            nc.scalar.dma_start(
                out=outp[:, 7 * RPC + SPL:8 * RPC], in_=ob7b[:]
            )
    nc.compile()
    return nc


def _get_nc():
    if "nc" not in _NC_CACHE:
        _NC_CACHE["nc"] = _build()
    return _NC_CACHE["nc"]


def _prep_in_maps(V, Wv, bv, Wo, bo, lq):
    Wv64 = np.asarray(Wv, np.float64)
    Wo64 = np.asarray(Wo, np.float64)
    bv64 = np.asarray(bv, np.float64)
    bo64 = np.asarray(bo, np.float64)

    # Fold per-head V-projection + output projection + attention mass (== Lq).
    Wo_r = Wo64.reshape(E, H, HD)                       # [n, h, b]
    W_eff = lq * np.einsum("ba,nhb->han", Wv64, Wo_r, optimize=True)
    W_eff = W_eff.reshape(E, E).astype(np.float32)      # [k, n]
    b_eff = (lq * np.einsum("nhb,b->n", Wo_r, bv64) + bo64).astype(np.float32)

    # lhsT block layout: wc[p, j*E + k*P + c] = W_eff[k*P + p, j*P + c]
    wc4 = W_eff.reshape(KT, P, JT, P)
    wc_all = np.ascontiguousarray(
        wc4.transpose(1, 2, 0, 3).reshape(P, JT * E)
    ).astype(BF16)
    # bank-0 weights at 1/W0S in E3M4, same per-k block layout
    w0_f8 = np.ascontiguousarray(
        (wc4[:, :, 0, :] / W0S).transpose(1, 0, 2).reshape(P, E)
    ).astype(F8E3)
    bias_blk = np.ascontiguousarray(b_eff.reshape(JT, P).T)   # [p, j] f32

    def wblk(j):
        return np.ascontiguousarray(wc_all[:, j * E:(j + 1) * E])

    common = {
        "cb": bias_blk,
        "c3": wblk(2), "c4": wblk(4), "c5": wblk(6),
        "d2": wblk(1), "d3": wblk(3), "d4": wblk(5), "d5": wblk(7),
    }
    X8 = np.asarray(V, dtype=np.float32).reshape(ROWS, E).astype(F8E3)
    in_maps = []
    for i in range(N_CORES):
        xsT = np.ascontiguousarray(X8[i * RPC:(i + 1) * RPC, :].T)  # [E, RPC]
        sl = lambda k: xsT[k * P:(k + 1) * P, :]
        c1_i = np.empty((P, 4 * P + RPC), F8E3)
        c1_i[:, 0:4 * P] = w0_f8[:, 0:4 * P]
        c1_i[:, 4 * P:] = sl(0)
        c2_i = np.ascontiguousarray(sl(1))
        d1_i = np.empty((P, 3 * RPC), F8E3)
        d1_i[:, 0:RPC] = w0_f8[:, RPC:]
        d1_i[:, RPC:2 * RPC] = sl(2)
        d1_i[:, 2 * RPC:] = sl(3)
        c6_i = np.ascontiguousarray(
            xsT[4 * P:8 * P].reshape(4, P, RPC).transpose(1, 0, 2).reshape(P, 4 * RPC)
        )
        m = dict(common)
        m.update({"c1": c1_i, "c2": c2_i, "c6": c6_i, "d1": d1_i})
        in_maps.append(m)
    return in_maps


def kernel(Q, K, V, Wq, bq, Wk, bk, Wv, bv, Wo, bo, **_unused):
    global LAST_RESULTS
    n, L, e = np.asarray(V).shape
    lq = float(np.asarray(Q).shape[1])
    in_maps = _prep_in_maps(V, Wv, bv, Wo, bo, lq)
    nc = _get_nc()
    LAST_RESULTS = run_bass_kernel_spmd(nc, in_maps, list(range(N_CORES)))
    parts = []
    for i in range(N_CORES):
        outp = LAST_RESULTS.results[i]["outp"]          # [P, JT*RPC] bf16
        oT = outp.reshape(P, JT, RPC).transpose(1, 0, 2).reshape(E, RPC)
        parts.append(np.ascontiguousarray(oT.T).astype(np.float32))
    out = np.concatenate(parts, axis=0)
    return np.ascontiguousarray(out).reshape(n, L, E)


# revision 25
# speedup vs baseline: 1.1232x; 1.1232x over previous
"""MultiHeadAttention kernel for 8x TRN2 NeuronCores.

The reference module's einsum reduces the attention tensor over BOTH the
query and key axes (attn_mass = sum_{q,k} softmax(logits)_k), and softmax
rows sum to 1, so attn_mass == Lq exactly for every (batch, head).  The
whole computation collapses to a single dense GEMM after folding the
(block-diagonal) per-head V-projection into the output projection:

    out = V_flat @ W_eff + b_eff          (4096 x 1024) @ (1024 x 1024)
    W_eff[h*hd+a, n] = Lq * sum_b Wv[b, a] * Wo[n, h*hd+b]
    b_eff[n]         = Lq * sum_{h,b} Wo[n, h*hd+b] * bv[b] + bo[n]

Row-sharded across 8 cores (512 rows each), computed TRANSPOSED so the
bias is a per-partition scalar folded into the PSUM eviction.

v5: X and bank-0's W stream as fp8 E3M4 (W0 pre-scaled by 1/8 to fit
E3M4's +-15.5 range; bank 0 evicts with a fused x8).  The PE upconverts
operands to fp22, so mixed bf16 x e3m4 matmuls are exact on the
quantized values (HW-verified); total quantization error vs the f64
reference: l2 1.24e-2, scale-absmax 1.44e-2 (gate 2e-2).  fp8 X halves
the bandwidth-critical early bytes -- the two HWDGE queues together
sustain only ~320 B/ns and the early phase (X + first W banks) bounds
when the PE can run dense.

NTFF-measured framework constants this schedule is built around:
window-start -> first dispatch ~1.1us, first bytes ~2.6us (sync) /
~3.5us (scalar), ~0.6us dispatch per DMA (engine-serial), completion
semaphore ~0.9us, teardown ring ~8.9us after the last output byte (all
fixed).  DMA needs >=2KB per-partition lines for full rate; the PE is
in-order so emission order must match operand arrival; Tile deps are
tile-granular (never split a psum bank's matmuls around its eviction).

  * sync:   c1=[W0 k0-3 | X0 | X1] (fp8) -> c2=[X4..X7] (fp8) -> bias
            -> W2 -> W4 -> W6     (~1.2MB)
    scalar: d1=[W0 k4-7 | X2 | X3] (fp8) -> W1 -> W3 -> W5 -> W7
  * emission: junk warm-up (HAM clock gate needs ~3.4us of PE busy),
    then b0 k-pieces as the three fp8 chunks land, then banks 1..7
    dense in W-arrival order; evictions (DVE, bias fused) pipeline
    behind each bank with output DMAs alternating queues;
  * tail: bank 7 stays full-width; its eviction splits DVE 288 /
    ACT 224 into separate ob tiles with DMAs on both queues.
"""

import numpy as np
import ml_dtypes

import concourse.bass as bass
import concourse.bacc as bacc
import concourse.mybir as mybir
from concourse.tile import TileContext
from concourse.bass_utils import run_bass_kernel_spmd

N_CORES = 8
E = 1024            # embed dim == d_model
H, HD = 16, 64      # heads, head dim
ROWS = 4096         # N * L = 2 * 2048
RPC = ROWS // N_CORES   # rows per core = 512
P = 128             # SBUF partitions
KT = E // P         # 8 contraction slabs
JT = E // P         # 8 output-column banks

N_WARM = 7          # junk matmuls before the first real matmul
JF = 512            # junk matmul free dim (warmup)
JF2 = 256           # filler junk free dim
SPL = 288           # bank-7 eviction DVE/ACT split point
W0S = 8.0           # bank-0 weight pre-scale (E3M4 max 15.5, |W0| < 99)

BF16 = ml_dtypes.bfloat16
F8E3 = ml_dtypes.float8_e3m4

_NC_CACHE = {}
LAST_RESULTS = None  # BassKernelResults of the most recent device run


def _build():
    f32 = mybir.dt.float32
    bf = mybir.dt.bfloat16
    e3 = mybir.dt.float8e3
    nc = bacc.Bacc(None, target_bir_lowering=False)

    def dram(name, cols, dt):
        return nc.declare_dram_parameter(name, [P, cols], dt, isOutput=False)

    # sync-queue stream (first-use order)
    c1 = dram("c1", 3 * RPC, e3)    # [W0 k0-3 (/8) | X0 | X1]
    c2 = dram("c2", 4 * RPC, e3)    # [X4 | X5 | X6 | X7]
    cb = dram("cb", JT, f32)        # bias
    c3 = dram("c3", E, bf)          # W2
    c4 = dram("c4", E, bf)          # W4
    c5 = dram("c5", E, bf)          # W6
    # scalar-queue stream
    d1 = dram("d1", 3 * RPC, e3)    # [W0 k4-7 (/8) | X2 | X3]
    d2 = dram("d2", E, bf)          # W1
    d3 = dram("d3", E, bf)          # W3
    d4 = dram("d4", E, bf)          # W5
    d5 = dram("d5", E, bf)          # W7
    outp = nc.declare_dram_parameter("outp", [P, JT * RPC], bf, isOutput=True)

    with TileContext(nc) as tc:
        with (
            tc.tile_pool(name="ip", bufs=1) as ip,
            tc.tile_pool(name="pp", bufs=1, space="PSUM") as pp,
            tc.tile_pool(name="op", bufs=1) as op,
        ):
            cap = nc.const_aps.aps[(bf, 1.0)]
            cb_l = cap.broadcast_to([P, P])
            cb_r = {JF: cap.broadcast_to([P, JF]), JF2: cap.broadcast_to([P, JF2])}

            cols = {"c1": 3 * RPC, "c2": 4 * RPC, "cb": JT, "c3": E,
                    "c4": E, "c5": E, "d1": 3 * RPC, "d2": E, "d3": E,
                    "d4": E, "d5": E}
            dts = {"c1": e3, "c2": e3, "cb": f32, "d1": e3}
            T = {}
            for prm in (c1, c2, cb, c3, c4, c5, d1, d2, d3, d4, d5):
                n = prm.name
                T[n] = ip.tile([P, cols[n]], dts.get(n, bf), name=n, tag=n)

            for prm in (c1, c2, cb, c3, c4, c5):
                nc.sync.dma_start(out=T[prm.name][:], in_=prm[:, :])
            for prm in (d1, d2, d3, d4, d5):
                nc.scalar.dma_start(out=T[prm.name][:], in_=prm[:, :])

            bias_t = T["cb"]
            WT = {1: T["d2"], 2: T["c3"], 3: T["d3"], 4: T["c4"],
                  5: T["d4"], 6: T["c5"], 7: T["d5"]}

            def rhs(k):
                if k < 2:
                    return T["c1"][:, (k + 1) * RPC:(k + 2) * RPC]
                if k < 4:
                    return T["d1"][:, (k - 1) * RPC:(k - 0) * RPC]
                return T["c2"][:, (k - 4) * RPC:(k - 3) * RPC]

            def lhsT(j, k):
                if j == 0:
                    t = T["c1"] if k < 4 else T["d1"]
                    return t[:, (k % 4) * P:(k % 4 + 1) * P]
                return WT[j][:, k * P:(k + 1) * P]

            ps = [
                pp.tile([P, RPC], f32, name=f"ps{j}", tag=f"ps{j}")
                for j in range(JT)
            ]
            obs = [
                op.tile([P, RPC], bf, name=f"ob{j}", tag=f"ob{j}")
                for j in range(JT - 1)
            ]
            ob7a = op.tile([P, SPL], bf, name="ob7a", tag="ob7a")
            ob7b = op.tile([P, RPC - SPL], bf, name="ob7b", tag="ob7b")

            def junk(i, f=JF2):
                nc.tensor.matmul(
                    ps[i][:, 0:f], cb_l, cb_r[f], start=True, stop=True,
                )

            for i in range(N_WARM):
                junk(7 - (i % 2), JF)   # ps7/ps6: real start comes latest

            def mm(j, k, start=False, stop=False):
                nc.tensor.matmul(
                    ps[j], lhsT(j, k), rhs(k), start=start, stop=stop,
                )

            def evict(j, eng):
                if j == 0:
                    # bank 0's W was pre-scaled by 1/W0S for E3M4 range
                    nc.vector.tensor_scalar(
                        obs[0][:], ps[0], W0S, bias_t[:, 0:1],
                        mybir.AluOpType.mult, mybir.AluOpType.add,
                    )
                else:
                    nc.vector.tensor_scalar_add(
                        obs[j][:], ps[j], bias_t[:, j:j + 1]
                    )
                eng.dma_start(
                    out=outp[:, j * RPC:(j + 1) * RPC], in_=obs[j][:]
                )

            # banks 0/1 interleaved following chunk arrivals: b0 k0-3
            # (c1/d1 land first), b1 k0-3 (W1 ~6.7us), then the c2-gated
            # k4-7 of b1 and b0 -- hides the X4-7 arrival wait behind
            # real work instead of a PE stall.
            mm(0, 0, start=True)
            mm(0, 1)
            junk(6)
            mm(0, 2)
            mm(0, 3)
            junk(6)
            mm(1, 0, start=True)
            mm(1, 1)
            mm(1, 2)
            mm(1, 3)
            for k in range(4, KT):
                mm(1, k, stop=(k == KT - 1))
            evict(1, nc.sync)
            for k in range(4, KT):
                mm(0, k, stop=(k == KT - 1))
            evict(0, nc.scalar)
            # banks 2..6 dense in W-arrival order
            engs = [nc.sync, nc.scalar]
            for n, j in enumerate([2, 3, 4, 5, 6]):
                mm(j, 0, start=True)
                for k in range(1, KT):
                    mm(j, k, stop=(k == KT - 1))
                evict(j, engs[n % 2])
            # bank 7 full-width; split eviction DVE/ACT, DMAs on both queues
            mm(7, 0, start=True)
            for k in range(1, KT):
                mm(7, k, stop=(k == KT - 1))
            nc.vector.tensor_scalar_add(
                ob7a[:], ps[7][:, 0:SPL], bias_t[:, 7:8]
            )
            nc.sync.dma_start(
                out=outp[:, 7 * RPC:7 * RPC + SPL], in_=ob7a[:]
            )
            nc.scalar.activation(
                ob7b[:],
                ps[7][:, SPL:RPC],
                mybir.ActivationFunctionType.Identity,
                bias=bias_t[:, 7:8],
            )
            nc.scalar.dma_start(
                out=outp[:, 7 * RPC + SPL:8 * RPC], in_=ob7b[:]
            )
    nc.compile()
    return nc


def _get_nc():
    if "nc" not in _NC_CACHE:
        _NC_CACHE["nc"] = _build()
    return _NC_CACHE["nc"]


def _prep_in_maps(V, Wv, bv, Wo, bo, lq):
    Wv64 = np.asarray(Wv, np.float64)
    Wo64 = np.asarray(Wo, np.float64)
    bv64 = np.asarray(bv, np.float64)
    bo64 = np.asarray(bo, np.float64)

    # Fold per-head V-projection + output projection + attention mass (== Lq).
    Wo_r = Wo64.reshape(E, H, HD)                       # [n, h, b]
    W_eff = lq * np.einsum("ba,nhb->han", Wv64, Wo_r, optimize=True)
    W_eff = W_eff.reshape(E, E).astype(np.float32)      # [k, n]
    b_eff = (lq * np.einsum("nhb,b->n", Wo_r, bv64) + bo64).astype(np.float32)

    # lhsT block layout: wc[p, j*E + k*P + c] = W_eff[k*P + p, j*P + c]
    wc4 = W_eff.reshape(KT, P, JT, P)
    wc_all = np.ascontiguousarray(
        wc4.transpose(1, 2, 0, 3).reshape(P, JT * E)
    ).astype(BF16)
    # bank-0 weights at 1/W0S in E3M4, same per-k block layout
    w0_f8 = np.ascontiguousarray(
        (wc4[:, :, 0, :] / W0S).transpose(1, 0, 2).reshape(P, E)
    ).astype(F8E3)
    bias_blk = np.ascontiguousarray(b_eff.reshape(JT, P).T)   # [p, j] f32

    def wblk(j):
        return np.ascontiguousarray(wc_all[:, j * E:(j + 1) * E])

    common = {
        "cb": bias_blk,
        "c3": wblk(2), "c4": wblk(4), "c5": wblk(6),
        "d2": wblk(1), "d3": wblk(3), "d4": wblk(5), "d5": wblk(7),
    }
    X8 = np.asarray(V, dtype=np.float32).reshape(ROWS, E).astype(F8E3)
    in_maps = []
    for i in range(N_CORES):
        xsT = np.ascontiguousarray(X8[i * RPC:(i + 1) * RPC, :].T)  # [E, RPC]
        sl = lambda k: xsT[k * P:(k + 1) * P, :]
        c1_i = np.empty((P, 3 * RPC), F8E3)
        c1_i[:, 0:RPC] = w0_f8[:, 0:RPC]
        c1_i[:, RPC:2 * RPC] = sl(0)
        c1_i[:, 2 * RPC:] = sl(1)
        d1_i = np.empty((P, 3 * RPC), F8E3)
        d1_i[:, 0:RPC] = w0_f8[:, RPC:]
        d1_i[:, RPC:2 * RPC] = sl(2)
        d1_i[:, 2 * RPC:] = sl(3)
        c2_i = np.ascontiguousarray(
            xsT[4 * P:8 * P].reshape(4, P, RPC).transpose(1, 0, 2).reshape(P, 4 * RPC)
        )
        m = dict(common)
        m.update({"c1": c1_i, "c2": c2_i, "d1": d1_i})
        in_maps.append(m)
    return in_maps


def kernel(Q, K, V, Wq, bq, Wk, bk, Wv, bv, Wo, bo, **_unused):
    global LAST_RESULTS
    n, L, e = np.asarray(V).shape
    lq = float(np.asarray(Q).shape[1])
    in_maps = _prep_in_maps(V, Wv, bv, Wo, bo, lq)
    nc = _get_nc()
    LAST_RESULTS = run_bass_kernel_spmd(nc, in_maps, list(range(N_CORES)))
    parts = []
    for i in range(N_CORES):
        outp = LAST_RESULTS.results[i]["outp"]          # [P, JT*RPC] bf16
        oT = outp.reshape(P, JT, RPC).transpose(1, 0, 2).reshape(E, RPC)
        parts.append(np.ascontiguousarray(oT.T).astype(np.float32))
    out = np.concatenate(parts, axis=0)
    return np.ascontiguousarray(out).reshape(n, L, E)


# revision 26
# speedup vs baseline: 1.1327x; 1.0084x over previous
"""MultiHeadAttention kernel for 8x TRN2 NeuronCores.

The reference module's einsum reduces the attention tensor over BOTH the
query and key axes (attn_mass = sum_{q,k} softmax(logits)_k), and softmax
rows sum to 1, so attn_mass == Lq exactly for every (batch, head).  The
whole computation collapses to a single dense GEMM after folding the
(block-diagonal) per-head V-projection into the output projection:

    out = V_flat @ W_eff + b_eff          (4096 x 1024) @ (1024 x 1024)
    W_eff[h*hd+a, n] = Lq * sum_b Wv[b, a] * Wo[n, h*hd+b]
    b_eff[n]         = Lq * sum_{h,b} Wo[n, h*hd+b] * bv[b] + bo[n]

Row-sharded across 8 cores (512 rows each), computed TRANSPOSED so the
bias is a per-partition scalar folded into the PSUM eviction.

v5: X and bank-0's W stream as fp8 E3M4 (W0 pre-scaled by 1/8 to fit
E3M4's +-15.5 range; bank 0 evicts with a fused x8).  The PE upconverts
operands to fp22, so mixed bf16 x e3m4 matmuls are exact on the
quantized values (HW-verified); total quantization error vs the f64
reference: l2 1.24e-2, scale-absmax 1.44e-2 (gate 2e-2).  fp8 X halves
the bandwidth-critical early bytes -- the two HWDGE queues together
sustain only ~320 B/ns and the early phase (X + first W banks) bounds
when the PE can run dense.

NTFF-measured framework constants this schedule is built around:
window-start -> first dispatch ~1.1us, first bytes ~2.6us (sync) /
~3.5us (scalar), ~0.6us dispatch per DMA (engine-serial), completion
semaphore ~0.9us, teardown ring ~8.9us after the last output byte (all
fixed).  DMA needs >=2KB per-partition lines for full rate; the PE is
in-order so emission order must match operand arrival; Tile deps are
tile-granular (never split a psum bank's matmuls around its eviction).

  * sync:   c1=[W0 k0-3 | X0 | X1] (fp8) -> c2=[X4..X7] (fp8) -> bias
            -> W2 -> W4 -> W6     (~1.2MB)
    scalar: d1=[W0 k4-7 | X2 | X3] (fp8) -> W1 -> W3 -> W5 -> W7
  * emission: junk warm-up (HAM clock gate needs ~3.4us of PE busy),
    then b0 k-pieces as the three fp8 chunks land, then banks 1..7
    dense in W-arrival order; evictions (DVE, bias fused) pipeline
    behind each bank with output DMAs alternating queues;
  * tail: bank 7 stays full-width; its eviction splits DVE 288 /
    ACT 224 into separate ob tiles with DMAs on both queues.
"""

import numpy as np
import ml_dtypes

import concourse.bass as bass
import concourse.bacc as bacc
import concourse.mybir as mybir
from concourse.tile import TileContext
from concourse.bass_utils import run_bass_kernel_spmd

N_CORES = 8
E = 1024            # embed dim == d_model
H, HD = 16, 64      # heads, head dim
ROWS = 4096         # N * L = 2 * 2048
RPC = ROWS // N_CORES   # rows per core = 512
P = 128             # SBUF partitions
KT = E // P         # 8 contraction slabs
JT = E // P         # 8 output-column banks

N_WARM = 7          # junk matmuls before the first real matmul
JF = 512            # junk matmul free dim (warmup)
JF2 = 256           # filler junk free dim
SPL = 288           # bank-7 eviction DVE/ACT split point
W0S = 8.0           # bank-0 weight pre-scale (E3M4 max 15.5, |W0| < 99)

BF16 = ml_dtypes.bfloat16
F8E3 = ml_dtypes.float8_e3m4

_NC_CACHE = {}
LAST_RESULTS = None  # BassKernelResults of the most recent device run


def _build():
    f32 = mybir.dt.float32
    bf = mybir.dt.bfloat16
    e3 = mybir.dt.float8e3
    nc = bacc.Bacc(None, target_bir_lowering=False)

    def dram(name, cols, dt):
        return nc.declare_dram_parameter(name, [P, cols], dt, isOutput=False)

    # sync-queue stream (first-use order)
    c1 = dram("c1", 3 * RPC, e3)    # [W0 k0-3 (/8) | X0 | X1]
    c2 = dram("c2", 4 * RPC, e3)    # [X4 | X5 | X6 | X7]
    cb = dram("cb", JT, f32)        # bias
    c3 = dram("c3", E, bf)          # W2
    c4 = dram("c4", E, bf)          # W4
    c5 = dram("c5", E, bf)          # W6
    # scalar-queue stream
    d1 = dram("d1", 3 * RPC, e3)    # [W0 k4-7 (/8) | X2 | X3]
    d2 = dram("d2", E, bf)          # W1
    d3 = dram("d3", E, bf)          # W3
    d4 = dram("d4", E, bf)          # W5
    d5 = dram("d5", E, bf)          # W7
    outp = nc.declare_dram_parameter("outp", [P, JT * RPC], bf, isOutput=True)

    with TileContext(nc) as tc:
        with (
            tc.tile_pool(name="ip", bufs=1) as ip,
            tc.tile_pool(name="pp", bufs=1, space="PSUM") as pp,
            tc.tile_pool(name="op", bufs=1) as op,
        ):
            cap = nc.const_aps.aps[(bf, 1.0)]
            cb_l = cap.broadcast_to([P, P])
            cb_r = {JF: cap.broadcast_to([P, JF]), JF2: cap.broadcast_to([P, JF2])}

            cols = {"c1": 3 * RPC, "c2": 4 * RPC, "cb": JT, "c3": E,
                    "c4": E, "c5": E, "d1": 3 * RPC, "d2": E, "d3": E,
                    "d4": E, "d5": E}
            dts = {"c1": e3, "c2": e3, "cb": f32, "d1": e3}
            T = {}
            for prm in (c1, c2, cb, c3, c4, c5, d1, d2, d3, d4, d5):
                n = prm.name
                T[n] = ip.tile([P, cols[n]], dts.get(n, bf), name=n, tag=n)

            for prm in (c1, c2, cb, c3, c4, c5):
                nc.sync.dma_start(out=T[prm.name][:], in_=prm[:, :])
            for prm in (d1, d2, d3, d4, d5):
                nc.scalar.dma_start(out=T[prm.name][:], in_=prm[:, :])

            bias_t = T["cb"]
            WT = {1: T["d2"], 2: T["c3"], 3: T["d3"], 4: T["c4"],
                  5: T["d4"], 6: T["c5"], 7: T["d5"]}

            def rhs(k):
                if k < 2:
                    return T["c1"][:, (k + 1) * RPC:(k + 2) * RPC]
                if k < 4:
                    return T["d1"][:, (k - 1) * RPC:(k - 0) * RPC]
                return T["c2"][:, (k - 4) * RPC:(k - 3) * RPC]

            def lhsT(j, k):
                if j == 0:
                    t = T["c1"] if k < 4 else T["d1"]
                    return t[:, (k % 4) * P:(k % 4 + 1) * P]
                return WT[j][:, k * P:(k + 1) * P]

            ps = [
                pp.tile([P, RPC], f32, name=f"ps{j}", tag=f"ps{j}")
                for j in range(JT)
            ]
            obs = [
                op.tile([P, RPC], bf, name=f"ob{j}", tag=f"ob{j}")
                for j in range(JT - 1)
            ]
            ob7a = op.tile([P, SPL], bf, name="ob7a", tag="ob7a")
            ob7b = op.tile([P, RPC - SPL], bf, name="ob7b", tag="ob7b")

            def junk(i, f=JF2):
                nc.tensor.matmul(
                    ps[i][:, 0:f], cb_l, cb_r[f], start=True, stop=True,
                )

            for i in range(N_WARM):
                junk(7 - (i % 2), JF)   # ps7/ps6: real start comes latest

            def mm(j, k, start=False, stop=False):
                nc.tensor.matmul(
                    ps[j], lhsT(j, k), rhs(k), start=start, stop=stop,
                )

            def evict(j, eng):
                if j == 0:
                    # bank 0's W was pre-scaled by 1/W0S for E3M4 range
                    nc.vector.tensor_scalar(
                        obs[0][:], ps[0], W0S, bias_t[:, 0:1],
                        mybir.AluOpType.mult, mybir.AluOpType.add,
                    )
                else:
                    nc.vector.tensor_scalar_add(
                        obs[j][:], ps[j], bias_t[:, j:j + 1]
                    )
                eng.dma_start(
                    out=outp[:, j * RPC:(j + 1) * RPC], in_=obs[j][:]
                )

            # banks 0/1 interleaved following chunk arrivals: b0 k0-3
            # (c1/d1 land first), b1 k0-3 (W1 ~6.7us), then the c2-gated
            # k4-7 of b1 and b0 -- hides the X4-7 arrival wait behind
            # real work instead of a PE stall.
            mm(0, 0, start=True)
            mm(0, 1)
            junk(6)
            mm(0, 2)
            mm(0, 3)
            junk(6)
            mm(1, 0, start=True)
            mm(1, 1)
            mm(1, 2)
            mm(1, 3)
            for k in range(4, KT):
                mm(1, k, stop=(k == KT - 1))
            evict(1, nc.sync)
            for k in range(4, KT):
                mm(0, k, stop=(k == KT - 1))
            evict(0, nc.scalar)
            # banks 2..6 dense in W-arrival order
            engs = [nc.sync, nc.scalar]
            for n, j in enumerate([2, 3, 4, 5, 6]):
                mm(j, 0, start=True)
                for k in range(1, KT):
                    mm(j, k, stop=(k == KT - 1))
                evict(j, engs[n % 2])
            # bank 7 full-width; eviction in two DVE pieces (ACT has a
            # ~0.6us start lag), DMAs on both queues
            mm(7, 0, start=True)
            for k in range(1, KT):
                mm(7, k, stop=(k == KT - 1))
            nc.vector.tensor_scalar_add(
                ob7a[:], ps[7][:, 0:SPL], bias_t[:, 7:8]
            )
            nc.sync.dma_start(
                out=outp[:, 7 * RPC:7 * RPC + SPL], in_=ob7a[:]
            )
            nc.vector.tensor_scalar_add(
                ob7b[:], ps[7][:, SPL:RPC], bias_t[:, 7:8]
            )
            nc.scalar.dma_start(
                out=outp[:, 7 * RPC + SPL:8 * RPC], in_=ob7b[:]
            )
    nc.compile()
    return nc


def _get_nc():
    if "nc" not in _NC_CACHE:
        _NC_CACHE["nc"] = _build()
    return _NC_CACHE["nc"]


def _prep_in_maps(V, Wv, bv, Wo, bo, lq):
    Wv64 = np.asarray(Wv, np.float64)
    Wo64 = np.asarray(Wo, np.float64)
    bv64 = np.asarray(bv, np.float64)
    bo64 = np.asarray(bo, np.float64)

    # Fold per-head V-projection + output projection + attention mass (== Lq).
    Wo_r = Wo64.reshape(E, H, HD)                       # [n, h, b]
    W_eff = lq * np.einsum("ba,nhb->han", Wv64, Wo_r, optimize=True)
    W_eff = W_eff.reshape(E, E).astype(np.float32)      # [k, n]
    b_eff = (lq * np.einsum("nhb,b->n", Wo_r, bv64) + bo64).astype(np.float32)

    # lhsT block layout: wc[p, j*E + k*P + c] = W_eff[k*P + p, j*P + c]
    wc4 = W_eff.reshape(KT, P, JT, P)
    wc_all = np.ascontiguousarray(
        wc4.transpose(1, 2, 0, 3).reshape(P, JT * E)
    ).astype(BF16)
    # bank-0 weights at 1/W0S in E3M4, same per-k block layout
    w0_f8 = np.ascontiguousarray(
        (wc4[:, :, 0, :] / W0S).transpose(1, 0, 2).reshape(P, E)
    ).astype(F8E3)
    bias_blk = np.ascontiguousarray(b_eff.reshape(JT, P).T)   # [p, j] f32

    def wblk(j):
        return np.ascontiguousarray(wc_all[:, j * E:(j + 1) * E])

    common = {
        "cb": bias_blk,
        "c3": wblk(2), "c4": wblk(4), "c5": wblk(6),
        "d2": wblk(1), "d3": wblk(3), "d4": wblk(5), "d5": wblk(7),
    }
    X8 = np.asarray(V, dtype=np.float32).reshape(ROWS, E).astype(F8E3)
    in_maps = []
    for i in range(N_CORES):
        xsT = np.ascontiguousarray(X8[i * RPC:(i + 1) * RPC, :].T)  # [E, RPC]
        sl = lambda k: xsT[k * P:(k + 1) * P, :]
        c1_i = np.empty((P, 3 * RPC), F8E3)
        c1_i[:, 0:RPC] = w0_f8[:, 0:RPC]
        c1_i[:, RPC:2 * RPC] = sl(0)
        c1_i[:, 2 * RPC:] = sl(1)
        d1_i = np.empty((P, 3 * RPC), F8E3)
        d1_i[:, 0:RPC] = w0_f8[:, RPC:]
        d1_i[:, RPC:2 * RPC] = sl(2)
        d1_i[:, 2 * RPC:] = sl(3)
        c2_i = np.ascontiguousarray(
            xsT[4 * P:8 * P].reshape(4, P, RPC).transpose(1, 0, 2).reshape(P, 4 * RPC)
        )
        m = dict(common)
        m.update({"c1": c1_i, "c2": c2_i, "d1": d1_i})
        in_maps.append(m)
    return in_maps


def kernel(Q, K, V, Wq, bq, Wk, bk, Wv, bv, Wo, bo, **_unused):
    global LAST_RESULTS
    n, L, e = np.asarray(V).shape
    lq = float(np.asarray(Q).shape[1])
    in_maps = _prep_in_maps(V, Wv, bv, Wo, bo, lq)
    nc = _get_nc()
    LAST_RESULTS = run_bass_kernel_spmd(nc, in_maps, list(range(N_CORES)))
    parts = []
    for i in range(N_CORES):
        outp = LAST_RESULTS.results[i]["outp"]          # [P, JT*RPC] bf16
        oT = outp.reshape(P, JT, RPC).transpose(1, 0, 2).reshape(E, RPC)
        parts.append(np.ascontiguousarray(oT.T).astype(np.float32))
    out = np.concatenate(parts, axis=0)
    return np.ascontiguousarray(out).reshape(n, L, E)
